# revision 34
# baseline (speedup 1.0000x reference)
"""CapsNet forward (conv1+relu, conv2, capsule transform + 3-iter dynamic
routing) on 8 TRN2 NeuronCores, pure data parallelism over the batch.

Layout notes (per core, B=64):
  conv1: im2col K=81 matmul, M=256 (2 chunks), N=(b,oh,ow).
  conv2: kernel-position decomposition: 81 positions x 2 in-halves
         accumulated in PSUM; out-channels PERMUTED so that partition
         c' = (c%8)*32 + c//8  (p-major) -> h2 halves by p.
  capsules: i = (ow*6+oh)*32 + g, p = c%8;  K-index m=(p,i).
  u' tile [128=(whl:4,g:32), (p:8, iblk:9, b:64)]  (i_loc = whl*32+g)
  routing kept in "i-layout": BT [128=i_loc, (iblk:9, j:10, b:64)].
"""

from contextlib import ExitStack

import numpy as np
import ml_dtypes

import concourse.bass as bass
import concourse.tile as tile
import concourse.masks as masks
from concourse import bacc, mybir
from concourse.bass_utils import run_bass_kernel_spmd

F32 = mybir.dt.float32
F32R = mybir.dt.float32r
BF16 = mybir.dt.bfloat16
AF = mybir.ActivationFunctionType
ALU = mybir.AluOpType
AX = mybir.AxisListType

N_CORES = 8
B_TOT = 512
BP = B_TOT // N_CORES      # 64 samples per core
NG = 2                     # conv1 sample-groups per core
GS = BP // NG              # 32 samples per group
NB_SUB = 4                 # N-chunks per conv2 half (8 samples each)

CONV_DT = F32R             # matmul dtype for conv1 (f32r: full-rate, ~tf32)

NOC = 10                   # out capsules
OUTD = 16                  # out dim (q)
NIB = 9                    # i-blocks of 128
JQ = NOC * OUTD            # 160

# sharded bf16 weight blob: w2 | ws | wy  (elems)
W2_N = 2 * 128 * 81 * 256          # 5,308,416
WS_N = 128 * 8 * NIB * JQ          # 1,474,560
WY_N = 16 * 720 * 128              # 1,474,560
WSY_N = WS_N + WY_N                # 2,949,120
BLOB_N = W2_N + WS_N + WY_N        # 8,257,536
SHARD_N = BLOB_N // N_CORES        # 1,032,192
W2_SH = W2_N // N_CORES            # 663,552
WSY_SH = WSY_N // N_CORES          # 368,640

_CACHE = {}


# ----------------------------------------------------------------- host prep
def _perm_c():
    # returns orig_of_perm: partition c' holds original channel (c'%32)*8+c'//32
    cp = np.arange(256)
    return (cp % 32) * 8 + cp // 32


def _host_prep(inputs):
    x = np.ascontiguousarray(inputs["input"].reshape(B_TOT, 28 * 28)).astype(
        ml_dtypes.bfloat16
    )
    w1 = np.ascontiguousarray(
        inputs["conv1_w"].reshape(256, 81).T
    ).astype(ml_dtypes.bfloat16)                                 # [81, 256]
    b1 = np.ascontiguousarray(
        inputs["conv1_b"].reshape(2, 128).T
    ).astype(np.float32)                                         # [128, 2]
    pc = _perm_c()
    w2p = np.asarray(inputs["conv2_w"])[pc]                      # [256oc', 256, 9, 9]
    w2 = np.ascontiguousarray(
        np.transpose(w2p, (1, 2, 3, 0)).reshape(2, 128, 81 * 256)
    ).astype(ml_dtypes.bfloat16)
    b2 = np.ascontiguousarray(
        np.asarray(inputs["conv2_b"])[pc].reshape(2, 128).T
    ).astype(np.float32)                                         # [128, 2]

    capw = np.asarray(inputs["cap_W"]).astype(np.float32)        # [1152,10,8,16]
    # Ws [128=i_loc, (p:8, iblk:9, jq:160)]
    ws = np.transpose(capw, (2, 0, 1, 3)).reshape(8, NIB, 128, JQ)
    ws = np.ascontiguousarray(np.transpose(ws, (2, 0, 1, 3))).reshape(
        128, 8 * NIB * JQ
    ).astype(ml_dtypes.bfloat16)
    # Wy [16=q, (iblk:9, j:10, p:8, i_loc:128)]  (chunk c = (iblk*10+j)*8+p)
    wq = np.transpose(capw, (3, 1, 2, 0)).reshape(16, NOC, 8, NIB, 128)
    wy = np.ascontiguousarray(
        np.transpose(wq, (0, 3, 1, 2, 4))
    ).reshape(16, 720 * 128).astype(ml_dtypes.bfloat16)

    w2f = w2.reshape(-1)
    wsyf = np.concatenate([ws.reshape(-1), wy.reshape(-1)])

    shared = {"w1": w1, "b1": b1, "b2": b2}
    maps = []
    for c in range(N_CORES):
        m = dict(shared)
        m["x"] = np.ascontiguousarray(x[c * BP : (c + 1) * BP])
        m["wsh"] = np.ascontiguousarray(
            np.concatenate(
                [w2f[c * W2_SH : (c + 1) * W2_SH],
                 wsyf[c * WSY_SH : (c + 1) * WSY_SH]]
            ).reshape(1, SHARD_N)
        )
        maps.append(m)
    return maps


# ------------------------------------------------------------------ IR build
def _emit(tc, nc, t, stage="full", reps=1):
    for _ in range(reps):
        _emit_once(tc, nc, t, stage)


def _conv2(tc, nc, w2pool, ps2, w2t_d, h1v, h2v, b2t):
    """conv2: 2 passes over 32-sample halves; kh-outer w2 stream."""
    for bh in range(2):
        cps = [
            [ps2.tile([128, 288], F32, tag=f"c2_{oh2}_{nbs}",
                      name=f"c2_{bh}_{oh2}_{nbs}")
             for nbs in range(NB_SUB)]
            for oh2 in range(2)
        ]
        for kh in range(9):
            w2t = w2pool.tile([128, 2 * 9 * 256], BF16, tag="w2")
            w2tv = w2t.rearrange("p (i c) -> p i c", i=2)
            nc.sync.dma_start(
                w2tv[:, :, :],
                bass.AP(
                    w2t_d,
                    kh * 9 * 256,
                    [[81 * 256, 128], [128 * 81 * 256, 2], [1, 9 * 256]],
                ),
            )
            for kw in range(9):
                for ih in range(2):
                    for oh2 in range(2):
                        lhsT = w2tv[
                            :, ih,
                            kw * 256 + oh2 * 128 : kw * 256 + (oh2 + 1) * 128,
                        ]
                        for nbs in range(NB_SUB):
                            b0 = bh * 32 + nbs * 8
                            rhs = h1v[
                                :, ih, b0 : b0 + 8,
                                kh : kh + 11 : 2, kw : kw + 11 : 2,
                            ].transpose([0, 1, 3, 2])
                            nc.tensor.matmul(
                                cps[oh2][nbs][:, :].rearrange(
                                    "p (b w) -> p b w", b=8
                                ),
                                lhsT,
                                rhs,
                                start=(kh == 0 and kw == 0 and ih == 0),
                                stop=(kh == 8 and kw == 8 and ih == 1),
                            )
        # h2 copy with bias (no relu), cast to bf16
        for oh2 in range(2):
            for nbs in range(NB_SUB):
                b0 = bh * 32 + nbs * 8
                nc.scalar.activation(
                    h2v[:, oh2, b0 : b0 + 8, :],
                    cps[oh2][nbs][:, :].rearrange("p (b w) -> p b w", b=8),
                    AF.Identity,
                    bias=b2t[:, oh2 : oh2 + 1],
                )


def _emit_once(tc, nc, t, stage="full"):
    """t: dict of DRAM APs."""
    if stage == "noop":
        with tc.tile_pool(name="np", bufs=1) as pool:
            z = pool.tile([64, 160], F32)
            nc.sync.dma_start(z[:, :], t["x"][:64, :160].bitcast(F32))
            nc.vector.tensor_scalar_mul(z[:, :], z[:, :], 0.0)
            nc.sync.dma_start(t["vout"][:, :], z[:, :])
        return
    ctx = ExitStack()
    # ---- persistent pools (span both phases)
    pers = ctx.enter_context(tc.tile_pool(name="pers", bufs=1))
    w1t = pers.tile([81, 256], BF16)
    b1t = pers.tile([128, 2], F32)
    b2t = pers.tile([128, 2], F32)
    eyet = pers.tile([128, 128], F32)
    # h2bf: [128=(pmod4:4,g:32) within half, (half:2, b:64, w:6, h:6)] bf16
    h2bf = pers.tile([128, 2 * BP * 36], BF16)
    upbf = pers.tile([128, 8 * NIB * BP], BF16)

    nc.sync.dma_start(w1t[:, :], t["w1"][:, :])
    nc.sync.dma_start(b1t[:, :], t["b1"][:, :])
    nc.sync.dma_start(b2t[:, :], t["b2"][:, :])
    masks.make_identity(nc, eyet[:, :])

    # ---- weight AllGather: each core ships 1/8 of (w2 | ws|wy) bf16
    dram = ctx.enter_context(tc.tile_pool(name="dram", bufs=1, space="DRAM"))
    agin = dram.tile([1, SHARD_N], BF16)
    w2full = dram.tile([1, W2_N], BF16, addr_space="Shared")
    wsyfull = dram.tile([1, WSY_N], BF16, addr_space="Shared")
    groups = [list(range(N_CORES))]
    nc.sync.dma_start(agin[:, :], t["wsh"][:, :])
    nc.gpsimd.collective_compute(
        "AllGather", ALU.bypass, replica_groups=groups,
        ins=[agin[:, 0:W2_SH].opt()],
        outs=[w2full[:, :].opt()],
    )
    nc.gpsimd.collective_compute(
        "AllGather", ALU.bypass, replica_groups=groups,
        ins=[agin[:, W2_SH:SHARD_N].opt()],
        outs=[wsyfull[:, :].opt()],
    )
    w2t_d = w2full[:, :].tensor
    wsy_d = wsyfull[:, :].tensor

    # ================= conv phase =================
    with tc.tile_pool(name="conv_sb", bufs=1) as csb:
        # h1 for the FULL per-core batch: [(ic), (ih:2, b:64, r:20, c:20)] bf16
        h1 = csb.tile([128, 2 * BP * 400], BF16)
        h1v = h1.rearrange("p (i b r c) -> p i b r c", i=2, b=BP, r=20)
        h2v = h2bf.rearrange("p (i b w) -> p i b w", i=2, b=BP)
        xs = t["x"]
        # ---- conv1 (all groups first; overlaps the w2 AllGather)
        with tc.tile_pool(name="im_sb", bufs=2) as imp, \
             tc.tile_pool(name="ps1", bufs=2, space="PSUM") as ps1:
            for g in range(NG):
                im = imp.tile([81, GS * 400], BF16, tag="im")
                for kh in range(9):
                    for kw in range(9):
                        k = kh * 9 + kw
                        src = bass.AP(
                            xs.tensor,
                            g * GS * 784 + kh * 28 + kw,
                            [[1, 1], [784, GS], [28, 20], [1, 20]],
                        )
                        nc.sync.dma_start(im[k : k + 1], src)
                # conv1 matmuls: K=81, M=2x128, N=GS*400 in chunks of 512
                nchunks = (GS * 400 + 511) // 512
                for mh in range(2):
                    for nb in range(nchunks):
                        n0 = nb * 512
                        n1 = min(n0 + 512, GS * 400)
                        pt = ps1.tile([128, 512], F32, tag="c1")
                        nc.tensor.matmul(
                            pt[:, : n1 - n0],
                            w1t[:, mh * 128 : (mh + 1) * 128],
                            im[:, n0:n1],
                            start=True,
                            stop=True,
                        )
                        nc.scalar.activation(
                            h1[:, (mh * BP + g * GS) * 400 + n0
                               : (mh * BP + g * GS) * 400 + n1],
                            pt[:, : n1 - n0],
                            AF.Relu,
                            bias=b1t[:, mh : mh + 1],
                        )
        # ---- conv2: 2 passes over 32-sample halves; kh-outer w2 stream
        with tc.tile_pool(name="w2_sb", bufs=2) as w2pool, \
             tc.tile_pool(name="ps2", bufs=1, space="PSUM") as ps2:
            _conv2(tc, nc, w2pool, ps2, w2t_d, h1v, h2v, b2t)

    # ---- u' build: 32 copies [32part, (iblk:9, b:64)]
    upbv = upbf.rearrange("p (k i b) -> p k i b", k=8, i=NIB)
    h2q = h2bf.rearrange("p (i b w h) -> p i b w h", i=2, b=BP, w=6)
    for p in range(8):
        half = p // 4
        pb = (p % 4) * 32
        for whl in range(4):
            src = (
                h2q[pb : pb + 32, half, :, :, :]
                .rearrange("p b w h -> p (w h) b")
                .rearrange("p (i l) b -> p i l b", l=4)[:, :, whl, :]
            )
            nc.vector.tensor_copy(upbv[whl * 32 : (whl + 1) * 32, p, :, :], src)

    if stage == "conv":
        updbg = pers.tile([128, 8 * NIB * BP], F32)
        nc.vector.tensor_copy(updbg[:, :], upbf[:, :])
        nc.sync.dma_start(t["dbg"][:, : 8 * NIB * BP], updbg[:, :])

    # ================= routing phase =================
    if stage == "conv":
        ctx.close()
        return
    with tc.tile_pool(name="rt", bufs=1) as rt, \
         tc.tile_pool(name="xw", bufs=2) as xw, \
         tc.tile_pool(name="mb", bufs=4) as mb, \
         tc.tile_pool(name="psr", bufs=1, space="PSUM") as psr:
        wst = rt.tile([128, 8 * NIB * JQ], BF16)
        nc.sync.dma_start(
            wst[:, :],
            bass.AP(wsy_d, 0, [[8 * NIB * JQ, 128], [1, 8 * NIB * JQ]]),
        )
        wsv = wst.rearrange("p (k i jq) -> p k i jq", k=8, i=NIB)

        BT = rt.tile([128, NIB * NOC * BP], F32)
        btv = BT.rearrange("p (i j b) -> p i j b", i=NIB, j=NOC)
        ebf = rt.tile([128, NIB * NOC * BP], BF16)
        ebv = ebf.rearrange("p (i j b) -> p i j b", i=NIB, j=NOC)
        zs = rt.tile([128, NIB * BP], F32)
        zsv = zs.rearrange("p (i b) -> p i b", i=NIB)
        rcb = rt.tile([128, NIB * BP], BF16)
        rcbv = rcb.rearrange("p (i b) -> p i b", i=NIB)
        rc = rt.tile([128, NIB * BP], F32)
        cT = rt.tile([128, NIB * NOC * BP], BF16)
        cTv = cT.rearrange("p (i j b) -> p i j b", i=NIB, j=NOC)

        vsp = rt.tile([64, NOC * OUTD], F32)       # [b, (j,q)]
        vv = vsp.rearrange("b (j q) -> b j q", j=NOC)
        vTs = rt.tile([16, NOC * BP], BF16)        # [q, (j, b)]
        vTv = vTs.rearrange("p (j b) -> p j b", j=NOC)
        sq = rt.tile([64, NOC], F32)
        sqa = rt.tile([64, NOC], F32)
        sqr = rt.tile([64, NOC], F32)
        coef = rt.tile([64, NOC], F32)
        epsb = rt.tile([64, 1], F32)
        nc.vector.memset(epsb[:, :], 1e-8)

        def squash_from_vspace():
            tmp = mb.tile([64, NOC * OUTD], F32, tag="sqt")
            nc.vector.tensor_tensor(tmp[:, :], vsp[:, :], vsp[:, :], ALU.mult)
            nc.vector.tensor_reduce(
                sq[:, :], tmp.rearrange("b (j q) -> b j q", j=NOC),
                AX.X, ALU.add,
            )
            nc.vector.tensor_scalar_add(sqa[:, :], sq[:, :], 1.0)
            nc.scalar.activation(sqr[:, :], sq[:, :], AF.Sqrt, bias=epsb[:, :])
            nc.vector.tensor_tensor(sqa[:, :], sqa[:, :], sqr[:, :], ALU.mult)
            nc.vector.reciprocal(coef[:, :], sqa[:, :])
            nc.vector.tensor_tensor(coef[:, :], coef[:, :], sq[:, :], ALU.mult)
            nc.vector.tensor_tensor(
                vv[:, :, :], vv[:, :, :],
                coef[:, :].unsqueeze(2).broadcast_to((64, NOC, OUTD)),
                ALU.mult,
            )

        def make_vT():
            for j in range(NOC):
                pt = psr.tile([16, BP], F32, tag="tp", name=f"ptv{j}")
                nc.tensor.transpose(pt[:, :], vv[:, j, :], eyet[:64, :64])
                nc.scalar.activation(vTv[:, j, :], pt[:, :], AF.Identity)

        def y_pass(first):
            """b-update: BT (=,+)= sum_p u'*y ; y from streamed Wy."""
            for iblk in range(NIB):
                wyi = xw.tile([16, 80 * 128], BF16, tag="wyi",
                              name=f"wyi{first}_{iblk}")
                nc.sync.dma_start(
                    wyi[:, :],
                    bass.AP(
                        wsy_d,
                        WS_N + iblk * 80 * 128,
                        [[720 * 128, 16], [1, 80 * 128]],
                    ),
                )
                for j in range(NOC):
                    yp = psr.tile([128, 8 * BP], F32, tag="yp")
                    ypv = yp.rearrange("p (k b) -> p k b", k=8)
                    for p in range(8):
                        lhsT = wyi[:, (j * 8 + p) * 128 : (j * 8 + p + 1) * 128]
                        rhs = vTv[:, j, :]
                        nc.tensor.matmul(
                            ypv[:, p, :], lhsT, rhs,
                            start=True, stop=True,
                        )
                    m = mb.tile([128, 8 * BP], BF16, tag="m")
                    mv = m.rearrange("p (k b) -> p k b", k=8)
                    nc.vector.tensor_tensor(
                        mv[:, :, :], ypv[:, :, :], upbv[:, :, iblk, :], ALU.mult
                    )
                    mr = m.rearrange("p (k b) -> p b k", k=8)
                    if first:
                        nc.vector.tensor_reduce(
                            btv[:, iblk, j, :], mr, AX.X, ALU.add
                        )
                    else:
                        tmp = mb.tile([128, BP], F32, tag="btmp")
                        nc.vector.tensor_reduce(tmp[:, :], mr, AX.X, ALU.add)
                        nc.vector.tensor_tensor(
                            btv[:, iblk, j, :], btv[:, iblk, j, :], tmp[:, :],
                            ALU.add,
                        )

        def softmax():
            nc.scalar.activation(ebf[:, :], BT[:, :], AF.Exp)
            nc.vector.tensor_reduce(
                zsv[:, :, :], ebv.transpose([0, 1, 3, 2]), AX.X, ALU.add
            )
            nc.vector.reciprocal(rc[:, :], zs[:, :])
            nc.vector.tensor_copy(rcb[:, :], rc[:, :])
            nc.vector.tensor_tensor(
                cTv[:, :, :, :], ebv[:, :, :, :],
                rcbv.unsqueeze(2).broadcast_to((128, NIB, NOC, BP)),
                ALU.mult,
            )

        def s_pass(iter1):
            if iter1:
                pa = psr.tile([128, BP], F32, tag="sp")
                pb = psr.tile([32, BP], F32, tag="sp2")
                k = 0
                for p in range(8):
                    for iblk in range(NIB):
                        rhs = upbv[:, p, iblk, :]
                        nc.tensor.matmul(
                            pa[:, :], wsv[:, p, iblk, 0:128], rhs,
                            start=(k == 0), stop=(k == 71),
                        )
                        nc.tensor.matmul(
                            pb[:, :], wsv[:, p, iblk, 128:160], rhs,
                            start=(k == 0), stop=(k == 71),
                        )
                        k += 1
                sa = rt.tile([128, BP], F32, tag="s1sa")
                sb = rt.tile([32, BP], F32, tag="s1sb")
                nc.scalar.activation(sa[:, :], pa[:, :], AF.Identity, scale=0.1)
                nc.scalar.activation(sb[:, :], pb[:, :], AF.Identity, scale=0.1)
                pta = psr.tile([64, 128], F32, tag="tp")
                nc.tensor.transpose(pta[:, :], sa[:, :], eyet[:, :])
                nc.scalar.activation(vsp[:, 0:128], pta[:, :], AF.Identity)
                ptb = psr.tile([64, 32], F32, tag="tp")
                nc.tensor.transpose(ptb[:, :], sb[:, :], eyet[:32, :32])
                nc.scalar.activation(vsp[:, 128:160], ptb[:, :], AF.Identity)
            else:
                for j in range(NOC):
                    x = xw.tile([128, 8 * NIB * BP], BF16, tag="x")
                    xv = x.rearrange("p (k i b) -> p k i b", k=8, i=NIB)
                    for iblk in range(NIB):
                        nc.vector.tensor_tensor(
                            xv[:, :, iblk, :],
                            upbv[:, :, iblk, :],
                            cTv[:, iblk, j, :].unsqueeze(1).broadcast_to(
                                (128, 8, BP)
                            ),
                            ALU.mult,
                        )
                    ps = psr.tile([16, BP], F32, tag="sp")
                    k = 0
                    for p in range(8):
                        for iblk in range(NIB):
                            nc.tensor.matmul(
                                ps[:, :],
                                wsv[:, p, iblk, j * 16 : (j + 1) * 16],
                                xv[:, p, iblk, :],
                                start=(k == 0), stop=(k == 71),
                            )
                            k += 1
                    sstg = rt.tile([16, BP], F32, tag="sstg")
                    nc.scalar.activation(sstg[:, :], ps[:, :], AF.Identity)
                    ptj = psr.tile([64, 16], F32, tag="tp")
                    nc.tensor.transpose(ptj[:, :], sstg[:, :], eyet[:16, :16])
                    nc.scalar.activation(vv[:, j, :], ptj[:, :], AF.Identity)

        # ---- iteration 1
        s_pass(iter1=True)
        squash_from_vspace()
        if stage == "s1":
            nc.sync.dma_start(t["vout"][:, :], vsp[:, :])
        if stage not in ("s1",):
            make_vT()
            if stage == "vt":
                vstg = rt.tile([128, NOC * BP], F32, name="vstg")
                nc.vector.tensor_copy(vstg[:, :], vTs[:, :])
                nc.sync.dma_start(t["dbg"][:, : NOC * BP], vstg[:, :])
            else:
                y_pass(first=True)
        if stage == "y1":
            nc.sync.dma_start(t["dbg"][:, : NIB * NOC * BP], BT[:, :])
        if stage not in ("s1", "y1"):
            # ---- iteration 2
            softmax()
            s_pass(iter1=False)
            squash_from_vspace()
            if stage == "it2":
                nc.sync.dma_start(t["vout"][:, :], vsp[:, :])
        if stage not in ("s1", "y1", "it2"):
            make_vT()
            y_pass(first=False)
            # ---- iteration 3
            softmax()
            s_pass(iter1=False)
            squash_from_vspace()
            # ---- output
            nc.sync.dma_start(t["vout"][:, :], vsp[:, :])

    ctx.close()


def _build(stage="full", reps=1):
    key = (stage, reps)
    if key in _CACHE:
        return _CACHE[key]
    nc = bacc.Bacc(
        "TRN2",
        target_bir_lowering=False,
        debug=False,
        enable_asserts=False,
        num_devices=N_CORES,
    )
    t = {}
    t["x"] = nc.dram_tensor("x", [BP, 784], BF16, kind="ExternalInput").ap()
    t["w1"] = nc.dram_tensor("w1", [81, 256], BF16, kind="ExternalInput").ap()
    t["b1"] = nc.dram_tensor("b1", [128, 2], F32, kind="ExternalInput").ap()
    t["b2"] = nc.dram_tensor("b2", [128, 2], F32, kind="ExternalInput").ap()
    t["wsh"] = nc.dram_tensor("wsh", [1, SHARD_N], BF16, kind="ExternalInput").ap()
    t["vout"] = nc.dram_tensor("vout", [64, 160], F32, kind="ExternalOutput").ap()
    if stage in ("conv", "y1", "vt", "y1a"):
        t["dbg"] = nc.dram_tensor(
            "dbg", [128, NIB * NOC * BP], F32, kind="ExternalOutput"
        ).ap()

    with tile.TileContext(nc) as tc:
        _emit(tc, nc, t, stage=stage, reps=reps)
    nc.compile()
    _CACHE[key] = nc
    return nc


LAST = None


def kernel(**inputs):
    global LAST
    nc = _build()
    maps = _host_prep(inputs)
    LAST = run_bass_kernel_spmd(nc, maps, list(range(N_CORES)))
    res = LAST.results
    out = np.concatenate(
        [np.asarray(res[c]["vout"]).reshape(BP, NOC, OUTD) for c in range(N_CORES)],
        axis=0,
    )
    return out.astype(np.float32)


if __name__ == "__main__":
    _build()
    print("build ok")

